# revision 1
# baseline (speedup 1.0000x reference)
"""Trainium2 Bass kernel for nn_KLFocalLossColBERT.

Reference computation (B=128, LQ=32, LD=256, D=128, NWAY=16, GAMMA=5):
  q  = l2norm(query_reps, axis=2)                     # over D
  d  = l2norm(doc_reps * doc_masks[..., None], axis=2)  # over Ld (token axis)
  sim = einsum('bqd,nbld->nbql', q, d)
  scores[b, n] = sum_q max_l sim
  logp = log_softmax(scores, -1); p = exp(logp); t = labels[:, :NWAY]
  loss = mean(exp(t) * (t - logp) * p**GAMMA)

Sharding: data-parallel over batch B across 8 cores (16 examples each).
Each core returns a [1,1] partial sum of loss entries; host sums / (B*NWAY).

Per-core pipeline per (b, n) pair:
  - DMA doc[n,b] [256,128] into SBUF as [128p, 2c, 128d] (l = c*128+p)
  - mask via per-partition tensor_scalar (maskT pre-transposed once on PE)
  - 2x PE transpose -> PSUM dT [128d, 256l]
  - DVE copy PSUM->SBUF; ACT Square+accum_out -> sumsq over l per feature d
  - rsqrt folded into the small qT operand (not the big doc tile)
  - PE matmul sim (4 docs packed via tile_position col-tiling) -> PSUM [128, 256]
  - one DVE reduce_max per 4-doc group -> staging column
Tail: ones-matmul -> scores, softmax/KL/focal on a [16,16] tile.
"""

import os
import sys

import numpy as np

for _p in ("/opt/trn_rl_repo", "/root/.axon_site/_ro/trn_rl_repo"):
    if os.path.isdir(_p) and _p not in sys.path:
        sys.path.insert(0, _p)

import concourse.bass as bass
import concourse.bacc as bacc_mod
import concourse.mybir as mybir
from concourse import bass_utils
from concourse.masks import make_identity
from concourse.tile import TileContext

F32 = mybir.dt.float32
I32 = mybir.dt.int32
AF = mybir.ActivationFunctionType
ALU = mybir.AluOpType

B, LQ, LD, D, NWAY = 128, 32, 256, 128, 16
GAMMA = 5
NCORES = 8
BL = B // NCORES  # 16 local examples per core

_nc_cache = None


def _build_nc():
    nc = bacc_mod.Bacc()
    q_d = nc.dram_tensor("q", [BL, LQ, D], F32, kind="ExternalInput")
    doc_d = nc.dram_tensor("doc", [NWAY, BL, LD, D], F32, kind="ExternalInput")
    msk_d = nc.dram_tensor("msk", [NWAY, BL, LD], I32, kind="ExternalInput")
    lab_d = nc.dram_tensor("lab", [BL, 2 * NWAY], F32, kind="ExternalInput")
    out_d = nc.dram_tensor("out", [1, 1], F32, kind="ExternalOutput")
    q_ap, doc_ap, msk_ap, lab_ap, out_ap = (
        q_d[:], doc_d[:], msk_d[:], lab_d[:], out_d[:]
    )

    with TileContext(nc) as tc:
        with (
            tc.tile_pool(name="consts", bufs=1) as consts,
            tc.tile_pool(name="apool", bufs=6) as apool,
            tc.tile_pool(name="rpool", bufs=20) as rpool,
            tc.tile_pool(name="scratch", bufs=2) as scratch,
            tc.tile_pool(name="small", bufs=4) as small,
            tc.tile_pool(name="qpool", bufs=3) as qpool,
            tc.tile_pool(name="ps_dt", bufs=3, space="PSUM") as ps_dt,
            tc.tile_pool(name="ps_sim", bufs=3, space="PSUM") as ps_sim,
            tc.tile_pool(name="ps_misc", bufs=2, space="PSUM") as ps_misc,
            tc.tile_pool(name="dram", bufs=1, space="DRAM") as dram,
        ):
            ident_g = consts.tile([128, 128], F32, tag="ident_g")
            make_identity(nc, ident_g)
            # re-materialize via DVE so PE matmuls wait on a single engine
            ident = consts.tile([128, 128], F32, tag="ident")
            nc.vector.tensor_copy(ident, ident_g)
            ones32 = consts.tile([32, 1], F32)
            nc.vector.memset(ones32, 1.0)
            ones16 = consts.tile([16, 1], F32)
            nc.vector.memset(ones16, 1.0)
            esel = consts.tile([128, 4], F32)
            nc.vector.memset(esel, 0.0)
            for k in range(4):
                nc.vector.memset(esel[32 * k:32 * k + 32, k:k + 1], 1.0)

            # ---- mask preload: [n, b, l] -> partitions (b%8)*16+n, group b//8
            mfs = []  # mf[g] [128 pairs, 256 l] f32
            for g in range(2):
                mi = consts.tile([128, LD], I32, tag=f"mi{g}")
                # partitions ordered (b_in_group, n); src iterates (b, n, l)
                src = msk_ap.rearrange("n (g b) l -> g b n l", g=2)[g]
                nc.sync.dma_start(out=mi, in_=src)
                mf = consts.tile([128, LD], F32, tag=f"mf{g}")
                nc.vector.tensor_copy(mf, mi)
                mfs.append(mf)
            # PE-transpose masks -> maskT[c][g] [128 l-in-chunk, 128 pairs]
            maskT = [[None, None], [None, None]]
            for g in range(2):
                for c in range(2):
                    pst = ps_misc.tile([128, 128], F32, tag="misc")
                    nc.tensor.transpose(pst, mfs[g][:, c * 128:(c + 1) * 128], ident)
                    mt = consts.tile([128, 128], F32, tag=f"mt{c}{g}")
                    nc.vector.tensor_copy(mt, pst)
                    maskT[c][g] = mt

            stage = consts.tile([128, BL * NWAY // 4], F32)  # 4 pairs/col

            for bl in range(BL):
                g, pgrp = bl // 8, (bl % 8) * 16

                # ---- q normalize + transpose (tiny)
                qn = qpool.tile([LQ, D], F32, tag="qn")
                nc.sync.dma_start(out=qn, in_=q_ap[bl])
                qsq = qpool.tile([LQ, D], F32, tag="qsq")
                qss = small.tile([LQ, 1], F32, tag="qss")
                nc.scalar.activation(qsq, qn, AF.Square, accum_out=qss)
                qnrm = small.tile([LQ, 1], F32, tag="qnrm")
                nc.scalar.activation(qnrm, qss, AF.Sqrt)
                qri = small.tile([LQ, 1], F32, tag="qri")
                nc.vector.reciprocal(qri, qnrm)
                qns = qpool.tile([LQ, D], F32, tag="qns")
                nc.vector.tensor_scalar_mul(qns, qn, qri)
                ps_qt = ps_misc.tile([D, LQ], F32, tag="misc")
                nc.tensor.transpose(ps_qt, qns, ident[:LQ, :LQ])
                qT = qpool.tile([D, LQ], F32, tag="qT")
                nc.vector.tensor_copy(qT, ps_qt)

                ssq = small.tile([128, NWAY], F32, tag="ssq")
                rtiles = []
                for n in range(NWAY):
                    # ---- load doc[n, bl] as [p, c, d], l = c*128 + p
                    A = apool.tile([128, 2, D], F32, tag="A")
                    nc.sync.dma_start(
                        out=A,
                        in_=doc_ap[n, bl].rearrange("(c p) d -> p c d", p=128),
                    )
                    # ---- mask (per-partition scalar per chunk)
                    Am = apool.tile([128, 2, D], F32, tag="Am")
                    for c in range(2):
                        nc.gpsimd.tensor_scalar_mul(
                            Am[:, c, :], A[:, c, :],
                            maskT[c][g][:, pgrp + n:pgrp + n + 1],
                        )
                    # ---- transpose both chunks into one PSUM tile [128d, 256l]
                    pdt = ps_dt.tile([D, LD], F32, tag="pdt")
                    for c in range(2):
                        nc.tensor.transpose(
                            pdt[:, c * 128:(c + 1) * 128], Am[:, c, :], ident
                        )
                    R = rpool.tile([D, LD], F32, tag="R")
                    if n % 2 == 0:
                        nc.vector.tensor_copy(R, pdt)
                    else:
                        nc.scalar.activation(R, pdt, AF.Copy)
                    # ---- sumsq over l per feature d (ACT square + accum)
                    sq = scratch.tile([D, LD], F32, tag="sq")
                    nc.scalar.activation(sq, pdt, AF.Square,
                                         accum_out=ssq[:, n:n + 1])
                    rtiles.append(R)

                # ---- batched rsqrt for all 16 n of this b
                nrm = small.tile([128, NWAY], F32, tag="nrm")
                nc.scalar.activation(nrm, ssq, AF.Sqrt)
                rinv = small.tile([128, NWAY], F32, tag="rinv")
                nc.vector.reciprocal(rinv, nrm)

                for gg in range(NWAY // 4):
                    psim = ps_sim.tile([128, LD], F32, tag="psim")
                    for k in range(4):
                        n = gg * 4 + k
                        qTs = qpool.tile([D, LQ], F32, tag="qTs")
                        nc.vector.tensor_scalar_mul(qTs, qT, rinv[:, n:n + 1])
                        nc.tensor.matmul(
                            psim[32 * k:32 * k + 32, :], lhsT=qTs,
                            rhs=rtiles[n], start=True, stop=True,
                            tile_position=(0, 32 * k),
                        )
                    jj = bl * 4 + gg
                    nc.vector.reduce_max(
                        stage[:, jj:jj + 1], psim, axis=mybir.AxisListType.X
                    )

            # ---- scores[1, 256] = ones32.T @ stage ; reshape to [16b, 16n]
            ps_sc = ps_misc.tile([4, BL * NWAY // 4], F32, tag="misc")
            nc.tensor.matmul(ps_sc, lhsT=esel, rhs=stage, start=True, stop=True)
            sc_row = small.tile([4, BL * NWAY // 4], F32, tag="scrow")
            nc.vector.tensor_copy(sc_row, ps_sc)
            dsc = dram.tile([4, BL, 4], F32, tag="dsc")
            nc.sync.dma_start(out=dsc, in_=sc_row.rearrange("k (b g) -> k b g", g=4))
            sc = small.tile([BL, NWAY], F32, tag="sc")
            nc.sync.dma_start(
                out=sc.rearrange("b (g k) -> b g k", k=4),
                in_=dsc.rearrange("k b g -> b g k"),
            )

            # ---- softmax / KL / focal tail on [16, 16]
            mrow = small.tile([BL, 1], F32, tag="mrow")
            nc.vector.reduce_max(mrow, sc, axis=mybir.AxisListType.X)
            xs = small.tile([BL, NWAY], F32, tag="xs")
            nc.vector.tensor_scalar(xs, sc, mrow, None, op0=ALU.subtract)
            ex = small.tile([BL, NWAY], F32, tag="ex")
            srow = small.tile([BL, 1], F32, tag="srow")
            nc.scalar.activation(ex, xs, AF.Exp, accum_out=srow)
            lgs = small.tile([BL, 1], F32, tag="lgs")
            nc.scalar.activation(lgs, srow, AF.Ln)
            logp = small.tile([BL, NWAY], F32, tag="logp")
            nc.vector.tensor_scalar(logp, xs, lgs, None, op0=ALU.subtract)
            rs = small.tile([BL, 1], F32, tag="rs")
            nc.vector.reciprocal(rs, srow)
            p = small.tile([BL, NWAY], F32, tag="p")
            nc.vector.tensor_scalar_mul(p, ex, rs)

            labt = small.tile([BL, NWAY], F32, tag="labt")
            nc.sync.dma_start(out=labt, in_=lab_ap[:, 0:NWAY])
            expt = small.tile([BL, NWAY], F32, tag="expt")
            nc.scalar.activation(expt, labt, AF.Exp)
            tml = small.tile([BL, NWAY], F32, tag="tml")
            nc.vector.tensor_tensor(tml, labt, logp, op=ALU.subtract)
            kl = small.tile([BL, NWAY], F32, tag="kl")
            nc.vector.tensor_tensor(kl, expt, tml, op=ALU.mult)
            p2 = small.tile([BL, NWAY], F32, tag="p2")
            nc.vector.tensor_tensor(p2, p, p, op=ALU.mult)
            p4 = small.tile([BL, NWAY], F32, tag="p4")
            nc.vector.tensor_tensor(p4, p2, p2, op=ALU.mult)
            p5 = small.tile([BL, NWAY], F32, tag="p5")
            nc.vector.tensor_tensor(p5, p4, p, op=ALU.mult)
            lv = small.tile([BL, NWAY], F32, tag="lv")
            nc.vector.tensor_tensor(lv, kl, p5, op=ALU.mult)
            rsum = small.tile([BL, 1], F32, tag="rsum")
            nc.vector.reduce_sum(rsum, lv, axis=mybir.AxisListType.X)
            ps_tot = ps_misc.tile([1, 1], F32, tag="misc")
            nc.tensor.matmul(ps_tot, lhsT=ones16, rhs=rsum, start=True, stop=True)
            ot = small.tile([1, 1], F32, tag="ot")
            nc.vector.tensor_copy(ot, ps_tot)
            nc.sync.dma_start(out=out_ap, in_=ot)

    nc.finalize()
    return nc


def _get_nc():
    global _nc_cache
    if _nc_cache is None:
        _nc_cache = _build_nc()
    return _nc_cache


def run(inputs, trace=False):
    q = np.ascontiguousarray(np.asarray(inputs["query_reps"], dtype=np.float32))
    doc = np.ascontiguousarray(np.asarray(inputs["doc_reps"], dtype=np.float32))
    msk = np.ascontiguousarray(np.asarray(inputs["doc_masks"], dtype=np.int32))
    lab = np.ascontiguousarray(np.asarray(inputs["labels"], dtype=np.float32))

    in_maps = []
    for k in range(NCORES):
        b0 = k * BL
        in_maps.append({
            "q": np.ascontiguousarray(q[b0:b0 + BL]),
            "doc": np.ascontiguousarray(doc[:, b0:b0 + BL]),
            "msk": np.ascontiguousarray(msk[:, b0:b0 + BL]),
            "lab": np.ascontiguousarray(lab[b0:b0 + BL]),
        })

    nc = _get_nc()
    res = bass_utils.run_bass_kernel_spmd(
        nc, in_maps, core_ids=list(range(NCORES)), trace=trace
    )
    total = np.float64(0.0)
    for r in res.results:
        total += np.float64(r["out"][0, 0])
    loss = np.float32(total / (B * NWAY))
    return np.array(loss, dtype=np.float32), res


def kernel(**inputs) -> np.ndarray:
    out, _ = run(inputs, trace=False)
    return out



# revision 8
# speedup vs baseline: 4.6448x; 4.6448x over previous
"""Trainium2 Bass kernel for nn_KLFocalLossColBERT.

Reference computation (B=128, LQ=32, LD=256, D=128, NWAY=16, GAMMA=5):
  q  = l2norm(query_reps, axis=2)                       # over D
  d  = l2norm(doc_reps * doc_masks[..., None], axis=2)  # over Ld (token axis)
  sim = einsum('bqd,nbld->nbql', q, d)
  scores[b, n] = sum_q max_l sim
  logp = log_softmax(scores, -1); p = exp(logp); t = labels[:, :NWAY]
  loss = mean(exp(t) * (t - logp) * p**GAMMA)

The graded metric is warm wall-clock of kernel(**inputs): the axon tunnel to
the 8 remote NeuronCores moves ~70-90 MB/s, so bytes-on-the-wire dominate.
Strategy:
  - shard the NWAY axis (2 docs/core): doc slices are contiguous, so the
    global fp8 array feeds jax sharding with zero host-side copies
  - pre-apply doc_masks into the fp8 cast on host (fused multiply+cast,
    threaded) -> no mask transfer, no on-device masking
  - pre-normalize + pre-transpose q on host -> qT [D, B*LQ] fp8, replicated
  - fp8(e4m3) wire format: end-to-end rel err 7.7e-4 vs 2e-2 tolerance
  - one cached jax.jit(shard_map(bass_exec)) built once per process
  - device kernel (per core, 2x128 (n,b) pairs): DMA fp8 doc tile, ACT
    upcast to bf16, 2x PE transpose -> PSUM [d, l]; ACT square+accum ->
    per-feature sumsq; rsqrt folded into the small qT operand; bf16 PE
    matmul sim + DVE reduce_max (4 pairs packed per PSUM tile); scores via
    ones-select matmul -> out [2, B]
  - softmax/KL/focal tail on host ([128,16], microseconds)
"""

import concurrent.futures as _cf
import os
import sys

import numpy as np

for _p in ("/opt/trn_rl_repo", "/root/.axon_site/_ro/trn_rl_repo"):
    if os.path.isdir(_p) and _p not in sys.path:
        sys.path.insert(0, _p)

import jax
import ml_dtypes
import concourse.bacc as bacc_mod
import concourse.mybir as mybir
from concourse import bass2jax
from concourse.masks import make_identity
from concourse.tile import TileContext
from jax.experimental.shard_map import shard_map
from jax.sharding import Mesh, PartitionSpec

F32 = mybir.dt.float32
BF16 = mybir.dt.bfloat16
FP8 = mybir.dt.float8e4
AF = mybir.ActivationFunctionType

B, LQ, LD, D, NWAY = 128, 32, 256, 128, 16
GAMMA = 5
NCORES = 8
NL = NWAY // NCORES  # 2 docs per core
NPAIR = NL * B       # 256 (n, b) pairs per core
NG = NPAIR // 4      # 64 groups of 4 pairs packed per PSUM tile

E4M3 = mybir.dt.np(FP8)  # ml_dtypes.float8_e4m3


def _build_nc():
    nc = bacc_mod.Bacc()
    doc_d = nc.dram_tensor("docm", [NL, B, LD, D], FP8, kind="ExternalInput")
    qt_d = nc.dram_tensor("qt", [D, B * LQ], FP8, kind="ExternalInput")
    out_d = nc.dram_tensor("out", [NL, B], F32, kind="ExternalOutput")
    doc_ap, qt_ap, out_ap = doc_d[:], qt_d[:], out_d[:]

    with TileContext(nc) as tc:
        with (
            tc.tile_pool(name="consts", bufs=1) as consts,
            tc.tile_pool(name="apool", bufs=4) as apool,
            tc.tile_pool(name="bpool", bufs=4) as bpool,
            tc.tile_pool(name="rpool", bufs=8) as rpool,
            tc.tile_pool(name="scratch", bufs=2) as scratch,
            tc.tile_pool(name="small", bufs=4) as small,
            tc.tile_pool(name="qpool", bufs=8) as qpool,
            tc.tile_pool(name="ps_dt", bufs=2, space="PSUM") as ps_dt,
            tc.tile_pool(name="ps_sim", bufs=2, space="PSUM") as ps_sim,
            tc.tile_pool(name="ps_misc", bufs=1, space="PSUM") as ps_misc,
        ):
            identb = consts.tile([128, 128], BF16, tag="identb")
            make_identity(nc, identb)
            # esel column k selects partition block [32k, 32k+32) (sum over q)
            esel = consts.tile([128, 4], F32)
            nc.vector.memset(esel, 0.0)
            for k in range(4):
                nc.vector.memset(esel[32 * k:32 * k + 32, k:k + 1], 1.0)

            # q^T for all b, loaded once: [128 d, 4096 (b q)] fp8 -> bf16
            qt8 = consts.tile([D, B * LQ], FP8, tag="qt8")
            nc.sync.dma_start(out=qt8, in_=qt_ap)
            qtb = consts.tile([D, B * LQ], BF16, tag="qtb")
            nc.scalar.activation(qtb, qt8, AF.Copy)

            stage = consts.tile([128, NG], F32)

            for g in range(NG):
                ssq = small.tile([128, 4], F32, tag="ssq")
                rtiles = []
                for k in range(4):
                    n, b = k >> 1, 2 * g + (k & 1)
                    # doc[n, b] as [p, c, d], l = c*128 + p (pre-masked fp8)
                    A = apool.tile([128, 2, D], FP8, tag="A")
                    nc.sync.dma_start(
                        out=A,
                        in_=doc_ap[n, b].rearrange("(c p) d -> p c d", p=128),
                    )
                    Ab = bpool.tile([128, 2, D], BF16, tag="Ab")
                    nc.scalar.activation(Ab, A, AF.Copy)
                    # transpose both chunks into one PSUM tile [128 d, 256 l]
                    # (bf16: fp8 values are exactly representable, lossless)
                    pdt = ps_dt.tile([D, LD], BF16, tag="pdt")
                    for c in range(2):
                        nc.tensor.transpose(
                            pdt[:, c * 128:(c + 1) * 128], Ab[:, c, :], identb
                        )
                    # per-feature sumsq over l (ACT square + free-axis accum)
                    sq = scratch.tile([D, LD], F32, tag="sq")
                    nc.scalar.activation(sq, pdt, AF.Square,
                                         accum_out=ssq[:, k:k + 1])
                    R = rpool.tile([D, LD], BF16, tag="R")
                    nc.vector.tensor_copy(R, pdt)
                    rtiles.append(R)

                nrm = small.tile([128, 4], F32, tag="nrm")
                nc.scalar.activation(nrm, ssq, AF.Sqrt)
                rinv = small.tile([128, 4], F32, tag="rinv")
                nc.vector.reciprocal(rinv, nrm)

                psim = ps_sim.tile([128, LD], F32, tag="psim")
                for k in range(4):
                    b = 2 * g + (k & 1)
                    qTs = qpool.tile([D, LQ], BF16, tag="qTs")
                    nc.vector.tensor_scalar_mul(
                        qTs, qtb[:, b * LQ:(b + 1) * LQ], rinv[:, k:k + 1]
                    )
                    nc.tensor.matmul(
                        psim[32 * k:32 * k + 32, :], lhsT=qTs, rhs=rtiles[k],
                        start=True, stop=True, tile_position=(0, 32 * k),
                    )
                nc.vector.reduce_max(
                    stage[:, g:g + 1], psim, axis=mybir.AxisListType.X
                )

            # scores: esel^T @ stage -> [4, NG]; sc[k, g] = score(n=k>>1,
            # b=2g+(k&1)); scatter to out[n, b] with one DMA per n
            ps_sc = ps_misc.tile([4, NG], F32, tag="misc")
            nc.tensor.matmul(ps_sc, lhsT=esel, rhs=stage, start=True, stop=True)
            sc_row = small.tile([4, NG], F32, tag="scrow")
            nc.vector.tensor_copy(sc_row, ps_sc)
            for n in range(NL):
                nc.sync.dma_start(
                    out=out_ap[n].rearrange("(g k1) -> k1 g", k1=2),
                    in_=sc_row[2 * n:2 * n + 2, :],
                )

    nc.finalize()
    return nc


_CACHE: dict = {}
_POOL = _cf.ThreadPoolExecutor(16)


def _get_runner():
    if "fn" in _CACHE:
        return _CACHE["fn"]

    bass2jax.install_neuronx_cc_hook()
    nc = _build_nc()

    partition_name = (
        nc.partition_id_tensor.name if nc.partition_id_tensor else None
    )
    in_names: list[str] = []
    out_names: list[str] = []
    out_avals: list[jax.core.ShapedArray] = []
    zero_outs: list[np.ndarray] = []
    for alloc in nc.m.functions[0].allocations:
        if not isinstance(alloc, mybir.MemoryLocationSet):
            continue
        name = alloc.memorylocations[0].name
        if alloc.kind == "ExternalInput":
            if name != partition_name:
                in_names.append(name)
        elif alloc.kind == "ExternalOutput":
            out_names.append(name)
            shape = tuple(alloc.tensor_shape)
            dtype = mybir.dt.np(alloc.dtype)
            out_avals.append(jax.core.ShapedArray(shape, dtype))
            zero_outs.append(np.zeros(shape, dtype))
    n_params = len(in_names)
    n_outs = len(out_avals)
    in_names = in_names + out_names
    if partition_name is not None:
        in_names.append(partition_name)
    donate = tuple(range(n_params, n_params + n_outs))

    def _body(*args):
        operands = list(args)
        if partition_name is not None:
            operands.append(bass2jax.partition_id_tensor())
        outs = bass2jax._bass_exec_p.bind(
            *operands,
            out_avals=tuple(out_avals),
            in_names=tuple(in_names),
            out_names=tuple(out_names),
            lowering_input_output_aliases=(),
            sim_require_finite=True,
            sim_require_nnan=True,
            nc=nc,
        )
        return tuple(outs)

    devices = jax.devices()[:NCORES]
    mesh = Mesh(np.asarray(devices), ("core",))
    # docm sharded over n (axis 0), qT replicated, out zeros sharded
    in_specs = (PartitionSpec("core"), PartitionSpec(), PartitionSpec("core"))
    out_specs = (PartitionSpec("core"),)
    sharded = jax.jit(
        shard_map(_body, mesh=mesh, in_specs=in_specs, out_specs=out_specs,
                  check_rep=False),
        donate_argnums=donate,
        keep_unused=True,
    )
    _CACHE["fn"] = sharded
    return sharded


def _masked_cast(doc: np.ndarray, msk: np.ndarray) -> np.ndarray:
    """(doc * mask).astype(e4m3), threaded over the NWAY axis."""
    out = np.empty(doc.shape, E4M3)
    m = msk.astype(np.float32)

    def work(n):
        out[n] = (doc[n] * m[n, :, :, None]).astype(E4M3)

    list(_POOL.map(work, range(NWAY)))
    return out


def _prep_q(q: np.ndarray) -> np.ndarray:
    """L2-normalize over D, transpose to [D, B*LQ], cast to e4m3."""
    nrm = np.sqrt((q.astype(np.float64) ** 2).sum(-1, keepdims=True))
    qn = (q / np.maximum(nrm, 1e-12)).astype(np.float32)
    qt = np.ascontiguousarray(qn.transpose(2, 0, 1).reshape(D, B * LQ))
    return qt.astype(E4M3)


def _tail(scores: np.ndarray, lab: np.ndarray) -> np.float32:
    """softmax / KL / focal on [B, NWAY] in float64."""
    sc = scores.astype(np.float64)
    m = sc.max(-1, keepdims=True)
    ls = np.log(np.exp(sc - m).sum(-1, keepdims=True)) + m
    logp = sc - ls
    p = np.exp(logp)
    t = lab[:, :NWAY].astype(np.float64)
    kl = np.exp(t) * (t - logp)
    lv = kl * p ** GAMMA
    return np.float32(lv.mean())


def run(inputs, trace=False):
    q = np.asarray(inputs["query_reps"], dtype=np.float32)
    doc = np.asarray(inputs["doc_reps"], dtype=np.float32)
    msk = np.asarray(inputs["doc_masks"])
    lab = np.asarray(inputs["labels"], dtype=np.float32)

    fn = _get_runner()
    docm8 = _masked_cast(doc, msk)
    qt8 = _prep_q(q)
    (outg,) = fn(docm8, qt8, np.zeros((NWAY, B), np.float32))
    scores = np.asarray(outg).astype(np.float32).T  # [B, NWAY]
    loss = _tail(scores, lab)

    class _Res:
        results = None
        instructions_and_trace = None
        profile_json = None
        exec_time_ns = None

    return np.array(loss, dtype=np.float32), _Res()


def kernel(**inputs) -> np.ndarray:
    out, _ = run(inputs)
    return out


# revision 16
# speedup vs baseline: 5.2373x; 1.1276x over previous
"""Trainium2 Bass kernel for nn_KLFocalLossColBERT.

Reference computation (B=128, LQ=32, LD=256, D=128, NWAY=16, GAMMA=5):
  q  = l2norm(query_reps, axis=2)                       # over D
  d  = l2norm(doc_reps * doc_masks[..., None], axis=2)  # over Ld (token axis)
  sim = einsum('bqd,nbld->nbql', q, d)
  scores[b, n] = sum_q max_l sim
  logp = log_softmax(scores, -1); p = exp(logp); t = labels[:, :NWAY]
  loss = mean(exp(t) * (t - logp) * p**GAMMA)

The graded metric is warm wall-clock of kernel(**inputs): the axon tunnel to
the 8 remote NeuronCores moves ~70-90 MB/s, so bytes-on-the-wire dominate.
Strategy:
  - shard the NWAY axis (2 docs/core): doc slices are contiguous, so the
    global fp8 arrays feed jax sharding with zero host-side copies
  - pre-apply doc_masks into the fp8 cast on host; fp8(e4m3) wire format:
    end-to-end rel err 7.9e-4 vs the 2e-2 tolerance (int4 tested: fails)
  - pipeline: doc is split into 8 slabs along B; each slab is cast on the
    main thread (f32 mul -> f16 SIMD cast -> 64K-entry LUT gather) and
    handed to an async device_put, so casting streams under the wire
  - pre-normalize + pre-transpose q on host -> qT [D, B*LQ] fp8, replicated
  - one cached jax.jit(shard_map(bass_exec)) built once per process
  - device kernel (per core, 2x128 (n,b) pairs): DMA fp8 doc tile, ACT
    upcast to bf16, 2x PE transpose -> PSUM [d, l]; ACT square+accum ->
    per-feature sumsq; rsqrt folded into the small qT operand; bf16 PE
    matmul sim + DVE reduce_max (4 pairs packed per PSUM tile); scores via
    ones-select matmul -> out [2, B]
  - softmax/KL/focal tail on host ([128,16], microseconds)
"""

import os
import sys

import numpy as np

for _p in ("/opt/trn_rl_repo", "/root/.axon_site/_ro/trn_rl_repo"):
    if os.path.isdir(_p) and _p not in sys.path:
        sys.path.insert(0, _p)

import jax
import ml_dtypes
import concourse.bacc as bacc_mod
import concourse.mybir as mybir
from concourse import bass2jax
from concourse.masks import make_identity
from concourse.tile import TileContext
from jax.experimental.shard_map import shard_map
from jax.sharding import Mesh, PartitionSpec

F32 = mybir.dt.float32
BF16 = mybir.dt.bfloat16
FP8 = mybir.dt.float8e4
AF = mybir.ActivationFunctionType

B, LQ, LD, D, NWAY = 128, 32, 256, 128, 16
GAMMA = 5
NCORES = 8
NL = NWAY // NCORES  # 2 docs per core
NPAIR = NL * B       # 256 (n, b) pairs per core
NG = NPAIR // 4      # 64 groups of 4 pairs packed per PSUM tile
NSLAB = 8            # doc pipeline slabs along B
BS = B // NSLAB      # 16 examples per slab

E4M3 = mybir.dt.np(FP8)  # ml_dtypes.float8_e4m3

# f16 bit pattern -> e4m3 byte lookup table (SIMD f32->f16, then gather)
_LUT = None


def _get_lut():
    global _LUT
    if _LUT is None:
        with np.errstate(invalid="ignore", over="ignore"):
            f16 = np.arange(65536, dtype=np.uint16).view(np.float16)
            _LUT = f16.astype(np.float32).astype(E4M3).view(np.uint8)
    return _LUT


def _build_nc():
    nc = bacc_mod.Bacc()
    doc_aps = []
    for kk in range(NSLAB):
        t = nc.dram_tensor(f"docm{kk}", [NL, BS, LD, D], FP8,
                           kind="ExternalInput")
        doc_aps.append(t[:])
    qt_d = nc.dram_tensor("qt", [D, B * LQ], FP8, kind="ExternalInput")
    out_d = nc.dram_tensor("out", [NL, B], F32, kind="ExternalOutput")
    qt_ap, out_ap = qt_d[:], out_d[:]

    with TileContext(nc) as tc:
        with (
            tc.tile_pool(name="consts", bufs=1) as consts,
            tc.tile_pool(name="apool", bufs=4) as apool,
            tc.tile_pool(name="bpool", bufs=4) as bpool,
            tc.tile_pool(name="rpool", bufs=8) as rpool,
            tc.tile_pool(name="scratch", bufs=2) as scratch,
            tc.tile_pool(name="small", bufs=4) as small,
            tc.tile_pool(name="qpool", bufs=8) as qpool,
            tc.tile_pool(name="ps_dt", bufs=2, space="PSUM") as ps_dt,
            tc.tile_pool(name="ps_sim", bufs=2, space="PSUM") as ps_sim,
            tc.tile_pool(name="ps_misc", bufs=1, space="PSUM") as ps_misc,
        ):
            identb = consts.tile([128, 128], BF16, tag="identb")
            make_identity(nc, identb)
            # esel column k selects partition block [32k, 32k+32) (sum over q)
            esel = consts.tile([128, 4], F32)
            nc.vector.memset(esel, 0.0)
            for k in range(4):
                nc.vector.memset(esel[32 * k:32 * k + 32, k:k + 1], 1.0)

            # q^T for all b, loaded once: [128 d, 4096 (b q)] fp8 -> bf16
            qt8 = consts.tile([D, B * LQ], FP8, tag="qt8")
            nc.sync.dma_start(out=qt8, in_=qt_ap)
            qtb = consts.tile([D, B * LQ], BF16, tag="qtb")
            nc.scalar.activation(qtb, qt8, AF.Copy)

            stage = consts.tile([128, NG], F32)

            for g in range(NG):
                ssq = small.tile([128, 4], F32, tag="ssq")
                rtiles = []
                for k in range(4):
                    n, b = k >> 1, 2 * g + (k & 1)
                    # doc[n, b] as [p, c, d], l = c*128 + p (pre-masked fp8)
                    A = apool.tile([128, 2, D], FP8, tag="A")
                    nc.sync.dma_start(
                        out=A,
                        in_=doc_aps[b // BS][n, b % BS].rearrange(
                            "(c p) d -> p c d", p=128
                        ),
                    )
                    Ab = bpool.tile([128, 2, D], BF16, tag="Ab")
                    nc.scalar.activation(Ab, A, AF.Copy)
                    # transpose both chunks into one PSUM tile [128 d, 256 l]
                    # (bf16: fp8 values are exactly representable, lossless)
                    pdt = ps_dt.tile([D, LD], BF16, tag="pdt")
                    for c in range(2):
                        nc.tensor.transpose(
                            pdt[:, c * 128:(c + 1) * 128], Ab[:, c, :], identb
                        )
                    # per-feature sumsq over l (ACT square + free-axis accum)
                    sq = scratch.tile([D, LD], F32, tag="sq")
                    nc.scalar.activation(sq, pdt, AF.Square,
                                         accum_out=ssq[:, k:k + 1])
                    R = rpool.tile([D, LD], BF16, tag="R")
                    nc.vector.tensor_copy(R, pdt)
                    rtiles.append(R)

                nrm = small.tile([128, 4], F32, tag="nrm")
                nc.scalar.activation(nrm, ssq, AF.Sqrt)
                rinv = small.tile([128, 4], F32, tag="rinv")
                nc.vector.reciprocal(rinv, nrm)

                psim = ps_sim.tile([128, LD], F32, tag="psim")
                for k in range(4):
                    b = 2 * g + (k & 1)
                    qTs = qpool.tile([D, LQ], BF16, tag="qTs")
                    nc.vector.tensor_scalar_mul(
                        qTs, qtb[:, b * LQ:(b + 1) * LQ], rinv[:, k:k + 1]
                    )
                    nc.tensor.matmul(
                        psim[32 * k:32 * k + 32, :], lhsT=qTs, rhs=rtiles[k],
                        start=True, stop=True, tile_position=(0, 32 * k),
                    )
                nc.vector.reduce_max(
                    stage[:, g:g + 1], psim, axis=mybir.AxisListType.X
                )

            # scores: esel^T @ stage -> [4, NG]; sc[k, g] = score(n=k>>1,
            # b=2g+(k&1)); scatter to out[n, b] with one DMA per n
            ps_sc = ps_misc.tile([4, NG], F32, tag="misc")
            nc.tensor.matmul(ps_sc, lhsT=esel, rhs=stage, start=True, stop=True)
            sc_row = small.tile([4, NG], F32, tag="scrow")
            nc.vector.tensor_copy(sc_row, ps_sc)
            for n in range(NL):
                nc.sync.dma_start(
                    out=out_ap[n].rearrange("(g k1) -> k1 g", k1=2),
                    in_=sc_row[2 * n:2 * n + 2, :],
                )

    nc.finalize()
    return nc


_CACHE: dict = {}


def _get_runner():
    if "fn" in _CACHE:
        return _CACHE["fn"]

    bass2jax.install_neuronx_cc_hook()
    nc = _build_nc()

    partition_name = (
        nc.partition_id_tensor.name if nc.partition_id_tensor else None
    )
    in_names: list[str] = []
    out_names: list[str] = []
    out_avals: list[jax.core.ShapedArray] = []
    zero_outs: list[np.ndarray] = []
    for alloc in nc.m.functions[0].allocations:
        if not isinstance(alloc, mybir.MemoryLocationSet):
            continue
        name = alloc.memorylocations[0].name
        if alloc.kind == "ExternalInput":
            if name != partition_name:
                in_names.append(name)
        elif alloc.kind == "ExternalOutput":
            out_names.append(name)
            shape = tuple(alloc.tensor_shape)
            dtype = mybir.dt.np(alloc.dtype)
            out_avals.append(jax.core.ShapedArray(shape, dtype))
            zero_outs.append(np.zeros(shape, dtype))
    n_params = len(in_names)
    n_outs = len(out_avals)
    in_names = in_names + out_names
    if partition_name is not None:
        in_names.append(partition_name)
    donate = tuple(range(n_params, n_params + n_outs))

    def _body(*args):
        operands = list(args)
        if partition_name is not None:
            operands.append(bass2jax.partition_id_tensor())
        outs = bass2jax._bass_exec_p.bind(
            *operands,
            out_avals=tuple(out_avals),
            in_names=tuple(in_names),
            out_names=tuple(out_names),
            lowering_input_output_aliases=(),
            sim_require_finite=True,
            sim_require_nnan=True,
            nc=nc,
        )
        return tuple(outs)

    devices = jax.devices()[:NCORES]
    mesh = Mesh(np.asarray(devices), ("core",))
    # doc slabs + out zeros sharded over n (axis 0), qT replicated
    in_specs = tuple(
        PartitionSpec() if nm == "qt" else PartitionSpec("core")
        for nm in in_names[: n_params + n_outs]
    )
    out_specs = (PartitionSpec("core"),)
    sharded = jax.jit(
        shard_map(_body, mesh=mesh, in_specs=in_specs, out_specs=out_specs,
                  check_rep=False),
        donate_argnums=donate,
        keep_unused=True,
    )
    from jax.sharding import NamedSharding

    _CACHE["fn"] = sharded
    _CACHE["shard"] = NamedSharding(mesh, PartitionSpec("core"))
    _CACHE["repl"] = NamedSharding(mesh, PartitionSpec())
    _CACHE["zeros"] = np.zeros((NWAY, B), np.float32)
    return sharded


def _cast_slab(dslab: np.ndarray, mslab: np.ndarray) -> np.ndarray:
    """(doc_slab * mask_slab) -> e4m3 via f16 + LUT gather (one B-slab)."""
    lut = _get_lut()
    t32 = dslab * mslab[..., None]
    t16 = t32.astype(np.float16)
    return lut[t16.view(np.uint16)].view(E4M3)


def _prep_q(q: np.ndarray) -> np.ndarray:
    """L2-normalize over D, transpose to [D, B*LQ], cast to e4m3."""
    nrm = np.sqrt((q.astype(np.float64) ** 2).sum(-1, keepdims=True))
    qn = (q / np.maximum(nrm, 1e-12)).astype(np.float32)
    qt = np.ascontiguousarray(qn.transpose(2, 0, 1).reshape(D, B * LQ))
    return qt.astype(E4M3)


def _tail(scores: np.ndarray, lab: np.ndarray) -> np.float32:
    """softmax / KL / focal on [B, NWAY] in float64."""
    sc = scores.astype(np.float64)
    m = sc.max(-1, keepdims=True)
    ls = np.log(np.exp(sc - m).sum(-1, keepdims=True)) + m
    logp = sc - ls
    p = np.exp(logp)
    t = lab[:, :NWAY].astype(np.float64)
    kl = np.exp(t) * (t - logp)
    lv = kl * p ** GAMMA
    return np.float32(lv.mean())


def run(inputs, trace=False):
    q = np.asarray(inputs["query_reps"], dtype=np.float32)
    doc = np.asarray(inputs["doc_reps"], dtype=np.float32)
    msk = np.asarray(inputs["doc_masks"])
    lab = np.asarray(inputs["labels"], dtype=np.float32)

    fn = _get_runner()
    # q first: its (replicated) transfer streams while slab 0 is cast
    qt_dev = jax.device_put(_prep_q(q), _CACHE["repl"])
    mskf = msk.astype(np.float32)
    slabs = []
    for kk in range(NSLAB):
        s8 = _cast_slab(
            doc[:, kk * BS:(kk + 1) * BS], mskf[:, kk * BS:(kk + 1) * BS]
        )
        slabs.append(jax.device_put(s8, _CACHE["shard"]))
    (outg,) = fn(*slabs, qt_dev, _CACHE["zeros"])
    scores = np.asarray(outg).astype(np.float32).T  # [B, NWAY]
    loss = _tail(scores, lab)

    class _Res:
        results = None
        instructions_and_trace = None
        profile_json = None
        exec_time_ns = None

    return np.array(loss, dtype=np.float32), _Res()


def kernel(**inputs) -> np.ndarray:
    out, _ = run(inputs)
    return out
